# revision 74
# baseline (speedup 1.0000x reference)
"""Trainium2 Bass kernel for the sparse submanifold 3D CNN (nn_Net_38963943309313).

Network: 7 blocks of 2 submanifold 3x3x3 convs on a 64^3 grid, 2x2x2 sparse
max-pools between blocks, channels 3->64->...->256, output [1,1,1,1,256].

Strategy (8 NeuronCores):
 - Shard z-slabs across cores for levels 0-2 (grids 64/32/16), AllGather the
   pooled activations (as bf16, split in halves so the first gather overlaps
   the back half of the producing level; z-padded gather buffers so per-core
   reads are dynamic-offset DMAs). Levels 3-6 (grids 8/4/2/1) are replicated
   on every core.
 - Convs are fp32r matmuls: activations channel-major [C, z, y, x] in SBUF
   (y/x zero-padded), 27 shifted-window matmuls accumulated in PSUM.
 - conv1 of block 0 runs in bf16 straight off a z-interleaved dense slab
   ([3*z+c, y, x] partition layout) with output-pair packing: one K=12
   matmul per (dy,dx) yields two adjacent output slices in PSUM rows
   0:64/64:128, exactly the duplicated ring layout conv2 needs (no copies).
 - 64-channel contractions (L0 conv2, L1 conv1) pack z-pairs into K=128 via
   duplicated storage; L0 conv2 additionally pairs two output slices into
   the two 64-column halves of the PE array.
 - Submanifold masking: conv1 evictions multiply by a broadcast bf16 mask
   (also zeroes the out-of-grid halo slices); conv2 evictions add
   (mask-1)*BIG so the following max-pool ignores inactive voxels; pool
   result is multiplied by the pooled mask.
 - Host->device traffic is minimized: weights ship as ONE bf16 blob sharded
   1/8 per core and AllGathered on device (converted to fp32 on device);
   only the small L0 weights are replicated so L0 can start immediately.
"""

import sys

sys.path.insert(0, "/opt/trn_rl_repo")

import numpy as np
import ml_dtypes
import concourse.bass as bass
import concourse.tile as tile
from concourse.tile import add_dep_helper
from concourse import bacc, mybir
from concourse.bass_utils import run_bass_kernel_spmd

NC = 8
GRID = 64
BIG = 1.0e30
CHANNELS = [(3, 64), (64, 64), (64, 96), (96, 96), (96, 128), (128, 128),
            (128, 160), (160, 160), (160, 192), (192, 192), (192, 224),
            (224, 224), (224, 256), (256, 256)]
F32 = mybir.dt.float32
F32R = mybir.dt.float32r
BF16 = mybir.dt.bfloat16
NPBF16 = ml_dtypes.bfloat16

OFFSETS = [(dz, dy, dx) for dz in (-1, 0, 1) for dy in (-1, 0, 1) for dx in (-1, 0, 1)]
# 9 (dy,dx) pairs for z-pair-packed layers
DYDX = [(dy, dx) for dy in (-1, 0, 1) for dx in (-1, 0, 1)]


def _ceil_div(a, b):
    return (a + b - 1) // b


# ---------------------------------------------------------------------------
# Weight blob layout (bf16, flat, AllGathered on device). L0 weights are NOT
# in the blob (shipped replicated so level 0 needn't wait for the gather).
# ---------------------------------------------------------------------------
def _kchunks(cin):
    return [min(128, cin - k0) for k0 in range(0, cin, 128)]


def blob_layout():
    """Ordered list of (name, shape) for the gathered weight blob."""
    ent = [
        ("w1p", (128, 9, 96)), ("w1l", (128, 9, 96)),
        ("w1c2_0", (96, 27, 96)),
        ("w2c1_0", (96, 27, 128)), ("w2c2_0", (128, 27, 128)),
        ("w3c1_0", (128, 27, 160)),
        ("w3c2_0", (128, 27, 160)), ("w3c2_1", (32, 27, 160)),
        ("w4c1_0", (128, 27, 192)), ("w4c1_1", (32, 27, 192)),
        ("w4c2_0", (128, 27, 192)), ("w4c2_1", (64, 27, 192)),
        ("w5c1_0", (128, 27, 224)), ("w5c1_1", (64, 27, 224)),
        ("w5c2_0", (128, 27, 224)), ("w5c2_1", (96, 27, 224)),
        ("w6c1_0", (128, 1, 256)), ("w6c1_1", (96, 1, 256)),
        ("w6c2_0", (128, 1, 256)), ("w6c2_1", (128, 1, 256)),
    ]
    offs = {}
    off = 0
    for name, sh in ent:
        n = int(np.prod(sh))
        offs[name] = (off, sh, n)
        off = off + ((n + 127) // 128) * 128
    chunk = _ceil_div(off, NC * 128) * 128
    return offs, chunk, NC * chunk


BLOB_OFF, BLOB_CHUNK, BLOB_TOTAL = blob_layout()


def build_host_inputs(features, coors, Ws):
    """All host-side data marshalling. Returns (in_maps, meta)."""
    z, y, x = coors[:, 0], coors[:, 1], coors[:, 2]
    dense = np.zeros((GRID, GRID, GRID, 3), np.float32)
    mask0 = np.zeros((GRID, GRID, GRID), np.float32)
    dense[z, y, x] = features  # last write wins (matches XLA CPU scatter)
    mask0[z, y, x] = 1.0

    # mask pyramid
    masks = [mask0]
    m = mask0
    for _ in range(6):
        mr = m.reshape(m.shape[0] // 2, 2, m.shape[1] // 2, 2, m.shape[2] // 2, 2)
        m = mr.max(axis=(1, 3, 5))
        masks.append(m)

    # ---- per-core dense slabs, z-interleaved partition layout ----
    # dense0[3*i + c, (yp*66 + xp)] = dense[8k-2+i, yp-1, xp-1, c]
    dense_slabs = []
    for k in range(NC):
        d = np.zeros((14, 3, 66, 66), np.float32)
        z0 = 8 * k - 2
        lo, hi = max(0, z0), min(GRID, z0 + 14)
        if hi > lo:
            d[lo - z0:hi - z0, :, 1:65, 1:65] = \
                dense[lo:hi].transpose(0, 3, 1, 2)
        dense_slabs.append(d.reshape(42, 66 * 66).astype(NPBF16))

    # ---- weight packs ----
    # conv1 of block 0, output-pair packed: [K=(dz,c)=12, j=(dy,dx)=9, 128].
    # Columns 0:64 produce out slice sl (taps = partitions 0:9), columns
    # 64:128 produce out slice sl+1 (taps = partitions 3:12); unused tap
    # rows are zero so both results are exact.
    w1zc = Ws[0].transpose(0, 3, 1, 2, 4).reshape(9, 9, 64)  # (dz,c),(dy,dx)
    w1zp = np.zeros((12, 9, 128), np.float32)
    w1zp[0:9, :, 0:64] = w1zc
    w1zp[3:12, :, 64:128] = w1zc
    w1zp = w1zp.astype(NPBF16)

    def pack_pair(W):  # [3,3,3,cin,co] -> pair [2*cin, 9, co] + left [cin, 9, co]
        cin, co = W.shape[3], W.shape[4]
        wp = np.zeros((2 * cin, 9, co), np.float32)
        wl = np.zeros((cin, 9, co), np.float32)
        for j, (dy, dx) in enumerate(DYDX):
            wp[0:cin, j] = W[0, dy + 1, dx + 1]      # dz=-1
            wp[cin:2 * cin, j] = W[1, dy + 1, dx + 1]  # dz=0
            wl[:, j] = W[2, dy + 1, dx + 1]          # dz=+1
        return wp, wl

    w0p, w0l = pack_pair(Ws[1])   # L0 conv2 64->64
    w1p, w1l = pack_pair(Ws[2])   # L1 conv1 64->96
    w0l = np.concatenate([w0l, w0l], axis=0)  # [128, 9, 64] both halves
    w1l = np.concatenate([w1l, w1l], axis=0)  # [128, 9, 96]

    def pack_generic(W):  # -> list of [kchunk, 27, co] arrays
        cin, co = W.shape[3], W.shape[4]
        wf = W.reshape(27, cin, co)
        out = []
        for k0 in range(0, cin, 128):
            kc = min(128, cin - k0)
            out.append(np.ascontiguousarray(
                wf[:, k0:k0 + kc, :].transpose(1, 0, 2)))  # [kc, 27, co]
        return out

    gen_w = {"w1p": w1p, "w1l": w1l}
    for li, wi in [("w1c2", 3), ("w2c1", 4), ("w2c2", 5), ("w3c1", 6),
                   ("w3c2", 7), ("w4c1", 8), ("w4c2", 9), ("w5c1", 10),
                   ("w5c2", 11)]:
        for ci, arr in enumerate(pack_generic(Ws[wi])):
            gen_w[f"{li}_{ci}"] = arr
    # L6: center tap only (1^3 grid)
    for li, wi in [("w6c1", 12), ("w6c2", 13)]:
        W = Ws[wi]
        cin = W.shape[3]
        wc = W[1, 1, 1]  # [cin, co]
        for ci, k0 in enumerate(range(0, cin, 128)):
            gen_w[f"{li}_{ci}"] = np.ascontiguousarray(
                wc[k0:k0 + min(128, cin - k0)][:, None, :])

    # ---- the bf16 blob + per-core chunks ----
    blob = np.zeros(BLOB_TOTAL, NPBF16)
    for name, (off, sh, n) in BLOB_OFF.items():
        blob[off:off + n] = gen_w[name].astype(NPBF16).reshape(-1)
    blob_chunks = [blob[k * BLOB_CHUNK:(k + 1) * BLOB_CHUNK] for k in range(NC)]

    # ---- per-core mask arrays ----
    def slab_mask(mask, z0, nsl):
        D2 = mask.shape[1] * mask.shape[2]
        out = np.zeros((nsl, D2), np.float32)
        for i in range(nsl):
            zg = z0 + i
            if 0 <= zg < mask.shape[0]:
                out[i] = mask[zg].reshape(-1)
        return out

    # L0 conv1-out multiply mask: slices [8k-1, 8k+10); row 10 stays zero
    # (the second half of the last conv1 output pair is halo garbage).
    m0mul = [np.concatenate([slab_mask(masks[0], 8 * k - 1, 10),
                             np.zeros((1, 4096), np.float32)]).astype(NPBF16)
             for k in range(NC)]
    # L0 maskneg for conv2-evict: [8, 4096] (0 / -BIG: exact in bf16)
    mn0 = [((masks[0][8 * k:8 * k + 8] - 1.0) * BIG).reshape(8, -1)
           .astype(NPBF16) for k in range(NC)]
    # L0 pool-out multiply: m1 on core's L1 slices [4, 1024]
    m1p = [masks[1][4 * k:4 * k + 4].reshape(4, -1).astype(np.float32)
           for k in range(NC)]
    # L1 conv1-evict multiply mask (m1 x ingrid): slices [4k-1, 4k+5)
    m1mul = [slab_mask(masks[1], 4 * k - 1, 6) for k in range(NC)]
    # L1 conv2-evict maskneg: slices [4k, 4k+4)
    mn1 = [((slab_mask(masks[1], 4 * k, 4) - 1.0) * BIG).astype(np.float32)
           for k in range(NC)]
    # L1 pool-out multiply: m2 on core's L2 slices [2, 256]
    m2p = [slab_mask(masks[2], 2 * k, 2) for k in range(NC)]
    # L2 conv1-evict multiply (m2 x ingrid): slices [2k-1, 2k+3)
    m2mul = [slab_mask(masks[2], 2 * k - 1, 4) for k in range(NC)]
    # L2 conv2-evict maskneg: slices [2k, 2k+2)
    mn2 = [((slab_mask(masks[2], 2 * k, 2) - 1.0) * BIG).astype(np.float32)
           for k in range(NC)]
    # L2 pool-out multiply: m3 on core's L3 slice [1, 64]
    m3p = [slab_mask(masks[3], k, 1) for k in range(NC)]
    # L3 (replicated): conv1-evict mul (m3 x ingrid) slices [-1, 9)
    m3mul_r = slab_mask(masks[3], -1, 10)
    mn3_r = ((slab_mask(masks[3], 0, 8) - 1.0) * BIG).astype(np.float32)
    m4p_r = slab_mask(masks[4], 0, 4)       # [4, 16]
    m4mul_r = slab_mask(masks[4], 0, 4)     # L4 out all valid (full grid)
    mn4_r = ((slab_mask(masks[4], 0, 4) - 1.0) * BIG).astype(np.float32)
    m5p_r = slab_mask(masks[5], 0, 2)
    m5mul_r = slab_mask(masks[5], 0, 2)
    mn5_r = ((slab_mask(masks[5], 0, 2) - 1.0) * BIG).astype(np.float32)
    m6p_r = slab_mask(masks[6], 0, 1)

    meta = {
        "mask_flags": {
            # whether the real mask (not just ingrid) has zeros at each level
            1: not np.all(masks[1] == 1.0),
            2: not np.all(masks[2] == 1.0),
            3: not np.all(masks[3] == 1.0),
            4: not np.all(masks[4] == 1.0),
            5: not np.all(masks[5] == 1.0),
            6: not np.all(masks[6] == 1.0),
        },
    }

    w0p_bf = w0p.astype(NPBF16)
    w0l_bf = w0l.astype(NPBF16)
    in_maps = []
    for k in range(NC):
        im = {
            "dense0": dense_slabs[k],
            "w1zp": w1zp, "w0p": w0p_bf, "w0l": w0l_bf,
            "wblob": blob_chunks[k],
            "m0mul": m0mul[k], "mn0": mn0[k], "m1p": m1p[k],
            "m1mul": m1mul[k], "mn1": mn1[k], "m2p": m2p[k],
            "m2mul": m2mul[k], "mn2": mn2[k], "m3p": m3p[k],
            "m3mul": m3mul_r, "mn3": mn3_r, "m4p": m4p_r,
            "m4mul": m4mul_r, "mn4": mn4_r, "m5p": m5p_r,
            "m5mul": m5mul_r, "mn5": mn5_r, "m6p": m6p_r,
        }
        in_maps.append(im)
    return in_maps, meta


def build_kernel(meta):
    import contextlib
    nc = bacc.Bacc("TRN2", target_bir_lowering=False, debug=False, num_devices=NC)
    mf = meta["mask_flags"]

    # ---------- DRAM I/O declarations ----------
    def din(name, shape, dt=F32):
        return nc.dram_tensor(name, list(shape), dt, kind="ExternalInput")

    dense0_d = din("dense0", (42, 4356), BF16)
    w1zp_d = din("w1zp", (12, 9, 128), BF16)
    w0p_d = din("w0p", (128, 9, 64), BF16)
    w0l_d = din("w0l", (128, 9, 64), BF16)
    wblob_d = din("wblob", (BLOB_CHUNK,), BF16)
    m0mul_d = din("m0mul", (11, 4096), BF16)
    mn0_d = din("mn0", (8, 4096), BF16); m1p_d = din("m1p", (4, 1024))
    m1mul_d = din("m1mul", (6, 1024)); mn1_d = din("mn1", (4, 1024))
    m2p_d = din("m2p", (2, 256))
    m2mul_d = din("m2mul", (4, 256)); mn2_d = din("mn2", (2, 256))
    m3p_d = din("m3p", (1, 64))
    m3mul_d = din("m3mul", (10, 64)); mn3_d = din("mn3", (8, 64))
    m4p_d = din("m4p", (4, 16)); m4mul_d = din("m4mul", (4, 16))
    mn4_d = din("mn4", (4, 16))
    m5p_d = din("m5p", (2, 4)); m5mul_d = din("m5mul", (2, 4))
    mn5_d = din("mn5", (2, 4)); m6p_d = din("m6p", (1, 1))

    out_d = nc.dram_tensor("out", [1, 256], F32, kind="ExternalOutput")
    import os as _os
    DBG = bool(_os.environ.get("K_DEBUG"))
    dbg_d = {}
    if DBG:
        for nm, sh in [("dP0", (64, 4, 1156)), ("dA1", (128, 8, 1156)),
                       ("dB1", (96, 6, 1156)), ("dC1", (96, 4, 1024)),
                       ("dP1", (96, 2, 324)), ("dA2", (96, 6, 324)),
                       ("dB2", (128, 4, 324)), ("dC2", (128, 2, 256)),
                       ("dP2", (128, 1, 100)), ("dA3", (128, 12, 100)),
                       ("dB3a", (128, 10, 100)), ("dC3a", (128, 512)),
                       ("dP4a", (128, 216)), ("dB4a", (128, 216)),
                       ("dC4a", (128, 64)), ("dP5a", (128, 64)),
                       ("dB5a", (128, 64)), ("dP6a", (128, 27)),
                       ("dX6a", (128, 1)), ("dC0", (64, 2, 4096))]:
            dbg_d[nm] = nc.dram_tensor(nm, list(sh), F32, kind="ExternalOutput")

    with tile.TileContext(nc) as tc:
        ctx = contextlib.ExitStack()
        with ctx:
            pst = ctx.enter_context(tc.tile_pool(name="ps", bufs=4, space="PSUM"))
            drm = ctx.enter_context(tc.tile_pool(name="dram", bufs=1, space="DRAM"))
            glob = ctx.enter_context(tc.tile_pool(name="glob", bufs=1))

            pid = nc.sync.partition_id()

            # const DRAM zeros for G-pad zeroing (avoids an SBUF zero tile
            # whose long-lived DMA reads confuse pool WAR tracking)
            zc = nc.inline_tensor(np.zeros((128, 1156), NPBF16), name="zconst")

            # DRAM gather buffers (activations gathered in bf16).
            # L0->L1 is gathered in two halves so the first AllGather
            # overlaps the second half of level-0 compute: G1a holds z1
            # slices {4k,4k+1}, G1b holds {4k+2,4k+3}, each with a 2-slice
            # zero pad block at both ends (block k at rows [2+2k, 4+2k)).
            cblob = drm.tile([BLOB_CHUNK], BF16)
            blobG = drm.tile([BLOB_TOTAL], BF16)
            # G1a: z1 {4k,4k+1} (2-slice blocks); G1b: z1 {4k+2}; G1c: z1
            # {4k+3} (1-slice blocks) — three gathers, launched as each
            # pooled slice lands so only G1c trails level 0.
            c1a_d = drm.tile([2, 64, 1156], BF16)
            c1b_d = drm.tile([1, 64, 1156], BF16)
            c1c_d = drm.tile([1, 64, 1156], BF16)
            G1a = drm.tile([20, 64, 1156], BF16)
            G1b = drm.tile([10, 64, 1156], BF16)
            G1c = drm.tile([10, 64, 1156], BF16)
            c2a_d = drm.tile([1, 96, 324], BF16)
            c2b_d = drm.tile([1, 96, 324], BF16)
            G2a = drm.tile([10, 96, 324], BF16)
            G2b = drm.tile([10, 96, 324], BF16)
            c3_d = drm.tile([1, 128, 100], BF16)
            G3 = drm.tile([12, 128, 100], BF16)
            gpad_insts = []
            wg = None

            def emit_comm_setup():
                # G-pad zeroing + weight-blob AllGather. Emitted after the
                # first conv so the small level-0 input DMAs win the queues.
                nonlocal wg
                for G, csz, npad in ((G1a, (64, 1156), 2),
                                     (G1b, (64, 1156), 1),
                                     (G1c, (64, 1156), 1),
                                     (G2a, (96, 324), 1), (G2b, (96, 324), 1),
                                     (G3, (128, 100), 2)):
                    n = G.shape[0]
                    for s in list(range(npad)) + list(range(n - npad, n)):
                        gpad_insts.append(
                            nc.sync.dma_start(G[s], zc[0:csz[0], 0:csz[1]]))
                # (collectives cannot read IO tensors; stage to internal DRAM)
                _cb = nc.sync.dma_start(cblob[:], wblob_d[:])
                wg = nc.gpsimd.collective_compute(
                    "AllGather", mybir.AluOpType.bypass,
                    replica_groups=[list(range(NC))],
                    ins=[cblob[:].opt()], outs=[blobG[:].opt()])
                add_dep_helper(wg.ins, _cb.ins, reason="chunk staged before gather")

            def wload_blob(pool, stage, name, dt=F32R):
                off, sh, n = BLOB_OFF[name]
                bf = stage.tile(list(sh), BF16, tag="wstage",
                                name=f"bf_{name}")
                dma = nc.sync.dma_start(
                    bf[:], blobG[off:off + n].rearrange(
                        "(p f) -> p f", p=sh[0]))
                add_dep_helper(dma.ins, wg.ins, reason="blob gather first")
                t = pool.tile(list(sh), dt, name=f"sb_{name}")
                nc.scalar.copy(t[:], bf[:])
                return t

            # persistent tail tensors (small; cross level boundaries)
            P4a = glob.tile([128, 216], F32); P4b = glob.tile([32, 216], F32)
            P5a = glob.tile([128, 64], F32); P5b = glob.tile([64, 64], F32)
            P6a = glob.tile([128, 27], F32); P6b = glob.tile([96, 27], F32)
            X6a = glob.tile([128, 1], F32); X6b = glob.tile([128, 1], F32)
            outt = glob.tile([128, 2], F32)
            for t in (P4a, P4b, P5a, P5b, P6a, P6b):
                nc.vector.memset(t[:].bitcast(F32), 0.0)

            # ================ LEVEL 0 ================
            # P0 spans L0..L1: level 1 reads the core's own pooled slices
            # directly from SBUF instead of waiting for the gather.
            st01 = contextlib.ExitStack()
            p0p = st01.enter_context(tc.tile_pool(name="p0p", bufs=1))
            P0 = p0p.tile([64, 4, 1156], BF16)
            with tc.tile_pool(name="l0w", bufs=1) as wp, \
                 tc.tile_pool(name="l0p", bufs=1) as pp, \
                 tc.tile_pool(name="l0s", bufs=2) as ss, \
                 tc.tile_pool(name="l0m", bufs=4) as sm:
                w1zp_t = wp.tile([12, 9, 128], BF16)
                nc.sync.dma_start(w1zp_t[:], w1zp_d[:])
                w0p_bf = wp.tile([128, 9, 64], BF16)
                w0l_bf = wp.tile([128, 9, 64], BF16)
                nc.sync.dma_start(w0p_bf[:], w0p_d[:])
                nc.sync.dma_start(w0l_bf[:], w0l_d[:])
                w0p_t = wp.tile([128, 9, 64], F32R)
                w0l_t = wp.tile([128, 9, 64], F32R)
                nc.scalar.copy(w0p_t[:], w0p_bf[:])
                nc.scalar.copy(w0l_t[:], w0l_bf[:])

                A0 = pp.tile([128, 4, 4356], F32R)
                C0 = pp.tile([64, 2, 4096], F32R)
                for _s in range(4):
                    nc.vector.memset(A0[:, _s, :].bitcast(F32), 0.0)
                nc.vector.memset(P0[:], 0.0)

                def l0_conv1(sl):
                    # Output-pair bf16 matmuls off the z-interleaved dense
                    # slab: PSUM rows 0:64 = h1[sl], rows 64:128 = h1[sl+1]
                    # (K=12 = 4 z-slices x 3 ch; zero weight rows pad each
                    # half). This writes ring slot sl%4 in the exact rows0/
                    # rows64 layout conv2's z-pair packing wants, so no dup
                    # copies are needed.
                    xs = ss.tile([12, 4356], BF16, tag="x0s")
                    nc.sync.dma_start(xs[:], dense0_d[3 * sl:3 * sl + 12, :])
                    win = xs[:].rearrange("p (a b) -> p a b", b=66)
                    mt = sm.tile([128, 4096], BF16, tag="m0mul", bufs=2)
                    nc.sync.dma_start(mt[0:64, :], m0mul_d[sl].unsqueeze(0)
                                      .to_broadcast((64, 4096)))
                    nc.sync.dma_start(mt[64:128, :],
                                      m0mul_d[sl + 1].unsqueeze(0)
                                      .to_broadcast((64, 4096)))
                    for chunk in range(8):
                        yb = chunk * 8
                        ps = pst.tile([128, 512], F32, tag="ps")
                        v = ps[:].rearrange("p (a b) -> p a b", b=64)
                        for j, (dy, dx) in enumerate(DYDX):
                            nc.tensor.matmul(
                                v, w1zp_t[:, j, :],
                                win[:, yb + 1 + dy:yb + 9 + dy,
                                    1 + dx:65 + dx],
                                start=(j == 0), stop=(j == 8))
                        d = A0[:, sl % 4, :].rearrange("p (a b) -> p a b", b=66)
                        nc.vector.tensor_mul(
                            d[:, yb + 1:yb + 9, 1:65], v,
                            mt[:, yb * 64:yb * 64 + 512].rearrange(
                                "p (a b) -> p a b", b=64))

                def l0_conv2(z):
                    # ring r: rows0 = h1[local r mod 4 writer], i.e.
                    # conv1(sl) wrote rows0@sl%4 and rows64@(sl-1)%4.
                    # out z needs h1 locals (z, z+1, z+2); out z+1 one more.
                    rA = z % 4         # rows0=h1[z], rows64=h1[z+1]
                    rB = (z + 1) % 4   # rows0=h1[z+1], rows64=h1[z+2]
                    rD = (z + 3) % 4   # rows0=h1[z+3]
                    mnt = {}
                    for zz, h in ((z, 0), (z + 1, 1)):
                        m = sm.tile([64, 4096], BF16, tag="mn0", bufs=2)
                        nc.sync.dma_start(m[:], mn0_d[zz].unsqueeze(0)
                                          .to_broadcast((64, 4096)))
                        mnt[h] = m
                    for chunk in range(8):
                        yb = chunk * 8
                        psA = pst.tile([64, 512], F32, tag="ps")
                        psB = pst.tile([64, 512], F32, tag="ps")
                        wA = A0[:, rA, :].rearrange("p (a b) -> p a b", b=66)
                        wB = A0[:, rB, :].rearrange("p (a b) -> p a b", b=66)
                        wD = A0[:, rD, :].rearrange("p (a b) -> p a b", b=66)
                        for j, (dy, dx) in enumerate(DYDX):
                            first, last = (j == 0), (j == 8)
                            ys = slice(yb + 1 + dy, yb + 9 + dy)
                            xsl = slice(1 + dx, 65 + dx)
                            vA = psA[:].rearrange("p (a b) -> p a b", b=64)
                            vB = psB[:].rearrange("p (a b) -> p a b", b=64)
                            # K=128 z-pair mms (full rows)
                            nc.tensor.matmul(vA, w0p_t[:, j, :],
                                             wA[:, ys, xsl],
                                             start=first, stop=False)
                            nc.tensor.matmul(vB, w0p_t[:, j, :],
                                             wB[:, ys, xsl],
                                             start=first, stop=False)
                            # K=64 leftovers, row-group paired:
                            # out z reads h1[z+2] at rows64 of rB;
                            # out z+1 reads h1[z+3] at rows0 of rD.
                            nc.tensor.matmul(vA, w0l_t[64:128, j, :],
                                             wB[64:128, ys, xsl],
                                             start=False, stop=last)
                            nc.tensor.matmul(vB, w0l_t[0:64, j, :],
                                             wD[0:64, ys, xsl],
                                             start=False, stop=last)
                        for ps_, zz, h in ((psA, z, 0), (psB, z + 1, 1)):
                            nc.vector.tensor_add(
                                C0[:, h, yb * 64:yb * 64 + 512], ps_[:],
                                mnt[h][:, yb * 64:yb * 64 + 512])

                def l0_pool(z):
                    zp = z // 2
                    nc.vector.tensor_max(C0[:, 0, :], C0[:, 0, :], C0[:, 1, :])
                    v = C0[:, 0, :].rearrange("p (a b) -> p a b", b=64)
                    t2 = ss.tile([64, 32, 64], F32R, tag="pool0b", bufs=1)
                    nc.vector.tensor_max(t2[:], v[:, 0::2, :], v[:, 1::2, :])
                    t3 = ss.tile([64, 32, 32], F32R, tag="pool0c", bufs=1)
                    nc.vector.tensor_max(t3[:], t2[:, :, 0::2], t2[:, :, 1::2])
                    mt = sm.tile([64, 1024], F32, tag="m1p", bufs=2)
                    nc.sync.dma_start(mt[:], m1p_d[zp].unsqueeze(0)
                                      .to_broadcast((64, 1024)))
                    dst = P0[:, zp, :].rearrange("p (a b) -> p a b", b=34)
                    nc.vector.tensor_mul(
                        dst[:, 1:33, 1:33], t3[:],
                        mt[:].rearrange("p (a b) -> p a b", b=32))

                ag1a = None
                for sl in range(10):
                    l0_conv1(sl)
                    if sl == 2:
                        # deferred so the first slabs' input DMAs win the
                        # queues over the 1.7MB blob staging + pad zeroing
                        emit_comm_setup()
                    if sl >= 3 and (sl - 3) % 2 == 0:
                        zz = sl - 3
                        l0_conv2(zz)
                        l0_pool(zz)
                    if sl == 5:
                        # P0 slices 0,1 complete: gather while L0 runs
                        _w = nc.sync.dma_start(
                            c1a_d[:].rearrange("z c v -> c z v"),
                            P0[:, 0:2, :])
                        ag1a = nc.gpsimd.collective_compute(
                            "AllGather", mybir.AluOpType.bypass,
                            replica_groups=[list(range(NC))],
                            ins=[c1a_d[:].opt()], outs=[G1a[2:18].opt()])
                        add_dep_helper(ag1a.ins, _w.ins, reason="c1a staged")
                        for gi in gpad_insts:
                            add_dep_helper(ag1a.ins, gi.ins,
                                           reason="G pads zeroed before gathers")
                    if sl == 7:
                        # P0 slice 2 complete
                        _w = nc.sync.dma_start(
                            c1b_d[:].rearrange("z c v -> c z v"),
                            P0[:, 2:3, :])
                        ag1b = nc.gpsimd.collective_compute(
                            "AllGather", mybir.AluOpType.bypass,
                            replica_groups=[list(range(NC))],
                            ins=[c1b_d[:].opt()], outs=[G1b[1:9].opt()])
                        add_dep_helper(ag1b.ins, _w.ins, reason="c1b staged")
                        for gi in gpad_insts:
                            add_dep_helper(ag1b.ins, gi.ins,
                                           reason="G pads zeroed before gathers")

                if DBG:
                    nc.sync.dma_start(dbg_d["dC0"][:], C0[:].bitcast(F32))
                nc.sync.dma_start(c1c_d[:].rearrange("z c v -> c z v"),
                                  P0[:, 3:4, :])

            # ---- AllGather L0 -> L1 (last slice) ----
            ag1c = nc.gpsimd.collective_compute(
                "AllGather", mybir.AluOpType.bypass,
                replica_groups=[list(range(NC))],
                ins=[c1c_d[:].opt()], outs=[G1c[1:9].opt()])
            for gi in gpad_insts:
                add_dep_helper(ag1c.ins, gi.ins, reason="G pads zeroed before gathers")

            # ================ LEVEL 1 ================
            with tc.tile_pool(name="l1w", bufs=1) as wp, \
                 tc.tile_pool(name="l1wst", bufs=2) as wst, \
                 tc.tile_pool(name="l1p", bufs=1) as pp, \
                 tc.tile_pool(name="l1s", bufs=2) as ss, \
                 tc.tile_pool(name="l1m", bufs=4) as sm:
                w1p_t = wload_blob(wp, wst, "w1p")
                w1l_t = wload_blob(wp, wst, "w1l")
                w1c2_t = wload_blob(wp, wst, "w1c2_0")

                A1 = pp.tile([128, 8, 1156], F32R)
                B1 = pp.tile([96, 6, 1156], F32R)
                C1 = pp.tile([96, 4, 1024], F32R)
                P1 = pp.tile([96, 2, 324], BF16)
                stb2 = pp.tile([64, 1, 1156], BF16)
                stb3 = pp.tile([64, 1, 1156], BF16)
                sta = pp.tile([64, 2, 1156], BF16)
                nc.vector.memset(B1[:].bitcast(F32), 0.0)
                nc.vector.memset(P1[:], 0.0)
                # A1 rows0 idx i = x1[4k-2+i], rows64 idx i = x1[4k-1+i].
                # idx 2..5 are the core's OWN pooled slices -> straight from
                # P0 in SBUF (no gather wait). True halos from the gathers:
                # idx 0 = core k-1 slice 2 (ag1b), idx 1 = k-1 slice 3
                # (ag1c, the only gather trailing L0), idx 6,7 = core k+1
                # slices 0,1 (ag1a, early).
                _rb = nc.sync.dma_start(
                    stb2[:], G1b[bass.ds(pid, 1)]
                    .rearrange("z c v -> c z v"))
                _rc = nc.sync.dma_start(
                    stb3[:], G1c[bass.ds(pid, 1)]
                    .rearrange("z c v -> c z v"))
                _ra = nc.sync.dma_start(
                    sta[:], G1a[bass.ds(pid * 2 + 4, 2)]
                    .rearrange("z c v -> c z v"))
                add_dep_helper(_rb.ins, ag1b.ins, reason="gather before dynamic read")
                add_dep_helper(_rc.ins, ag1c.ins, reason="gather before dynamic read")
                add_dep_helper(_ra.ins, ag1a.ins, reason="gather before dynamic read")
                nc.scalar.copy(A1[0:64, 2:6, :], P0[:])
                nc.vector.tensor_copy(A1[64:128, 1:5, :], P0[:])
                nc.scalar.copy(A1[0:64, 6:8, :], sta[:])
                nc.vector.tensor_copy(A1[64:128, 5:7, :], sta[:])
                nc.scalar.copy(A1[0:64, 0:1, :], stb2[:])

                def l1_conv1(sl):
                    # A1 rows0 idx i = x1[4k-2+i]; rows64 idx i = x1[4k-1+i].
                    # out sl (global 4k-1+sl): pair = A1[:, sl] (dz=-1,0);
                    # leftover dz=+1 = rows64 idx sl+1 == rows0 idx sl+2.
                    mt = sm.tile([96, 1024], F32, tag="m1mul")
                    nc.sync.dma_start(mt[:], m1mul_d[sl].unsqueeze(0)
                                      .to_broadcast((96, 1024)))
                    pss = [pst.tile([96, 512], F32, tag="ps", name=f"ps_l1_{sl}_{_c}") for _c in range(2)]
                    wA = A1[:, sl, :].rearrange("p (a b) -> p a b", b=34)
                    wB = A1[64:128, sl + 1, :].rearrange("p (a b) -> p a b", b=34)
                    wC = A1[0:64, sl + 2, :].rearrange("p (a b) -> p a b", b=34)
                    for j, (dy, dx) in enumerate(DYDX):
                        xsl = slice(1 + dx, 33 + dx)
                        for chunk in range(2):
                            yb = chunk * 16
                            ys = slice(yb + 1 + dy, yb + 17 + dy)
                            nc.tensor.matmul(
                                pss[chunk][:].rearrange("p (a b) -> p a b", b=32),
                                w1p_t[:, j, :], wA[:, ys, xsl],
                                start=(j == 0), stop=False)
                        # row-paired leftovers: chunk0 on rows 64:128,
                        # chunk1 on rows 0:64 (concurrent row groups)
                        ys0 = slice(1 + dy, 17 + dy)
                        ys1 = slice(17 + dy, 33 + dy)
                        nc.tensor.matmul(
                            pss[0][:].rearrange("p (a b) -> p a b", b=32),
                            w1l_t[64:128, j, :], wB[:, ys0, xsl],
                            start=False, stop=(j == 8))
                        nc.tensor.matmul(
                            pss[1][:].rearrange("p (a b) -> p a b", b=32),
                            w1l_t[0:64, j, :], wC[:, ys1, xsl],
                            start=False, stop=(j == 8))
                    for chunk in range(2):
                        yb = chunk * 16
                        dst = B1[:, sl, :].rearrange("p (a b) -> p a b", b=34)
                        nc.vector.tensor_mul(
                            dst[:, yb + 1:yb + 17, 1:33],
                            pss[chunk][:].rearrange("p (a b) -> p a b", b=32),
                            mt[:, yb * 32:yb * 32 + 512].rearrange(
                                "p (a b) -> p a b", b=32))

                def l1_conv2(sl):
                    mt = sm.tile([96, 1024], F32, tag="mn1")
                    nc.sync.dma_start(mt[:], mn1_d[sl].unsqueeze(0)
                                      .to_broadcast((96, 1024)))
                    for chunk in range(2):
                        yb = chunk * 16
                        ps = pst.tile([96, 512], F32, tag="ps")
                        for o, (dz, dy, dx) in enumerate(OFFSETS):
                            w = B1[:, sl + 1 + dz, :].rearrange(
                                "p (a b) -> p a b", b=34)
                            nc.tensor.matmul(
                                ps[:].rearrange("p (a b) -> p a b", b=32),
                                w1c2_t[:, o, :],
                                w[:, yb + 1 + dy:yb + 17 + dy, 1 + dx:33 + dx],
                                start=(o == 0), stop=(o == 26))
                        nc.vector.tensor_add(C1[:, sl, yb * 32:yb * 32 + 512],
                                             ps[:],
                                             mt[:, yb * 32:yb * 32 + 512])

                def l1_pool(zz):
                    zp = zz // 2
                    nc.vector.tensor_max(C1[:, zz, :], C1[:, zz, :], C1[:, zz + 1, :])
                    v = C1[:, zz, :].rearrange("p (a b) -> p a b", b=32)
                    t2 = ss.tile([96, 16, 32], F32R, tag="pool1b")
                    nc.vector.tensor_max(t2[:], v[:, 0::2, :], v[:, 1::2, :])
                    t3 = ss.tile([96, 16, 16], F32R, tag="pool1c")
                    nc.vector.tensor_max(t3[:], t2[:, :, 0::2], t2[:, :, 1::2])
                    mt = sm.tile([96, 256], F32, tag="m2p")
                    nc.sync.dma_start(mt[:], m2p_d[zp].unsqueeze(0)
                                      .to_broadcast((96, 256)))
                    dst = P1[:, zp, :].rearrange("p (a b) -> p a b", b=18)
                    nc.vector.tensor_mul(
                        dst[:, 1:17, 1:17], t3[:],
                        mt[:].rearrange("p (a b) -> p a b", b=16))

                # Slices 2..5 depend only on local data (and the early ag1a
                # halo for 4,5); 0,1 need the late ag1b halo, so they run
                # last. conv2(z)/pool fire as soon as their inputs exist;
                # pool(2) therefore completes mid-level and its gather
                # (P1 slice 1, the z2-odd half) overlaps the rest of L1.
                ag2_odd = None
                done, conv2_done = set(), set()
                for sl in (2, 3, 4, 5, 0, 1):
                    if sl == 0:
                        # copies off the last-arriving gather (ag1c)
                        nc.scalar.copy(A1[0:64, 1:2, :], stb3[:])
                        nc.vector.tensor_copy(A1[64:128, 0:1, :], stb3[:])
                    l1_conv1(sl)
                    done.add(sl)
                    for z in range(4):
                        if z in conv2_done or not {z, z + 1, z + 2} <= done:
                            continue
                        l1_conv2(z)
                        conv2_done.add(z)
                        if z == 3:
                            l1_pool(2)
                            _w = nc.sync.dma_start(
                                c2a_d[:].rearrange("z c v -> c z v"),
                                P1[:, 1:2, :])
                            ag2_odd = nc.gpsimd.collective_compute(
                                "AllGather", mybir.AluOpType.bypass,
                                replica_groups=[list(range(NC))],
                                ins=[c2a_d[:].opt()], outs=[G2b[1:9].opt()])
                            add_dep_helper(ag2_odd.ins, _w.ins,
                                           reason="c2a staged")
                            for gi in gpad_insts:
                                add_dep_helper(ag2_odd.ins, gi.ins,
                                               reason="G pads zeroed before gathers")
                        if z == 1:
                            l1_pool(0)

                if DBG:
                    nc.sync.dma_start(dbg_d["dA1"][:], A1[:].bitcast(F32))
                    nc.sync.dma_start(dbg_d["dB1"][:], B1[:].bitcast(F32))
                    nc.sync.dma_start(dbg_d["dC1"][:], C1[:].bitcast(F32))
                nc.sync.dma_start(c2b_d[:].rearrange("z c v -> c z v"),
                                  P1[:, 0:1, :])

            st01.close()
            # ---- AllGather L1 -> L2 (z2-even half) ----
            ag2_even = nc.gpsimd.collective_compute(
                "AllGather", mybir.AluOpType.bypass,
                replica_groups=[list(range(NC))],
                ins=[c2b_d[:].opt()], outs=[G2a[1:9].opt()])
            for gi in gpad_insts:
                add_dep_helper(ag2_even.ins, gi.ins, reason="G pads zeroed before gathers")

            # ================ LEVEL 2 ================
            with tc.tile_pool(name="l2w", bufs=1) as wp, \
                 tc.tile_pool(name="l2wst", bufs=2) as wst, \
                 tc.tile_pool(name="l2p", bufs=1) as pp, \
                 tc.tile_pool(name="l2s", bufs=2) as ss, \
                 tc.tile_pool(name="l2m", bufs=4) as sm:
                w2c1_t = wload_blob(wp, wst, "w2c1_0")
                w2c2_t = wload_blob(wp, wst, "w2c2_0")
                A2 = pp.tile([96, 6, 324], F32R)
                B2 = pp.tile([128, 4, 324], F32R)
                C2 = pp.tile([128, 2, 256], F32R)
                P2 = pp.tile([128, 1, 100], BF16)
                A2st = pp.tile([96, 6, 324], BF16)
                nc.vector.memset(B2[:].bitcast(F32), 0.0)
                nc.vector.memset(P2[:], 0.0)
                # window [2k-2, 2k+4): alternating G2a/G2b, blocks k-1..k+1
                for i in range(6):
                    G2h = G2a if i % 2 == 0 else G2b
                    agh = ag2_even if i % 2 == 0 else ag2_odd
                    _r3 = nc.sync.dma_start(
                        A2st[:, i:i + 1, :],
                        G2h[bass.ds(pid + i // 2, 1)]
                        .rearrange("z c v -> c z v"))
                    add_dep_helper(_r3.ins, agh.ins,
                                   reason="gather before dynamic read")
                nc.scalar.copy(A2[:], A2st[:])

                for s0 in (0, 2):
                    ps = pst.tile([128, 512], F32, tag="ps")
                    for o, (dz, dy, dx) in enumerate(OFFSETS):
                        w = A2[:].rearrange("p z (a b) -> p z a b", b=18)
                        nc.tensor.matmul(
                            ps[:].rearrange("p (z a b) -> p z a b", z=2, a=16),
                            w2c1_t[:, o, :],
                            w[:, s0 + dz + 1:s0 + dz + 3,
                              1 + dy:17 + dy, 1 + dx:17 + dx],
                            start=(o == 0), stop=(o == 26))
                    mt = sm.tile([128, 512], F32, tag="m2mul")
                    nc.sync.dma_start(
                        mt[:], m2mul_d[s0:s0 + 2].flatten().unsqueeze(0)
                        .to_broadcast((128, 512)))
                    dst = B2[:].rearrange("p z (a b) -> p z a b", b=18)
                    nc.vector.tensor_mul(
                        dst[:, s0:s0 + 2, 1:17, 1:17],
                        ps[:].rearrange("p (z a b) -> p z a b", z=2, a=16),
                        mt[:].rearrange("p (z a b) -> p z a b", z=2, a=16))

                ps = pst.tile([128, 512], F32, tag="ps")
                for o, (dz, dy, dx) in enumerate(OFFSETS):
                    w = B2[:].rearrange("p z (a b) -> p z a b", b=18)
                    nc.tensor.matmul(
                        ps[:].rearrange("p (z a b) -> p z a b", z=2, a=16),
                        w2c2_t[:, o, :],
                        w[:, dz + 1:dz + 3, 1 + dy:17 + dy, 1 + dx:17 + dx],
                        start=(o == 0), stop=(o == 26))
                if mf[2]:
                    mt = sm.tile([128, 512], F32, tag="mn2")
                    nc.sync.dma_start(mt[:], mn2_d[:].flatten().unsqueeze(0)
                                      .to_broadcast((128, 512)))
                    nc.vector.tensor_add(C2[:].rearrange("p a b -> p (a b)"),
                                         ps[:], mt[:])
                else:
                    nc.scalar.copy(C2[:].rearrange("p a b -> p (a b)"), ps[:])

                # L2 pool
                nc.vector.tensor_max(C2[:, 0, :], C2[:, 0, :], C2[:, 1, :])
                v = C2[:, 0, :].rearrange("p (a b) -> p a b", b=16)
                t2 = ss.tile([128, 8, 16], F32R, tag="pool2b")
                nc.vector.tensor_max(t2[:], v[:, 0::2, :], v[:, 1::2, :])
                dst = P2[:, 0, :].rearrange("p (a b) -> p a b", b=10)
                if mf[3]:
                    t3 = ss.tile([128, 8, 8], F32R, tag="pool2c")
                    nc.vector.tensor_max(t3[:], t2[:, :, 0::2], t2[:, :, 1::2])
                    mt = sm.tile([128, 64], F32, tag="m3p")
                    nc.sync.dma_start(mt[:], m3p_d[0].unsqueeze(0)
                                      .to_broadcast((128, 64)))
                    nc.vector.tensor_mul(
                        dst[:, 1:9, 1:9], t3[:],
                        mt[:].rearrange("p (a b) -> p a b", b=8))
                else:
                    nc.vector.tensor_max(dst[:, 1:9, 1:9],
                                         t2[:, :, 0::2], t2[:, :, 1::2])

                if DBG:
                    nc.sync.dma_start(dbg_d["dA2"][:], A2[:].bitcast(F32))
                    nc.sync.dma_start(dbg_d["dB2"][:], B2[:].bitcast(F32))
                    nc.sync.dma_start(dbg_d["dC2"][:], C2[:].bitcast(F32))
                nc.sync.dma_start(c3_d[:].rearrange("z c v -> c z v"), P2[:])

            # ---- AllGather L2 -> L3 ----
            ag3 = nc.gpsimd.collective_compute(
                "AllGather", mybir.AluOpType.bypass,
                replica_groups=[list(range(NC))],
                ins=[c3_d[:].opt()], outs=[G3[2:10].opt()])
            for gi in gpad_insts:
                add_dep_helper(ag3.ins, gi.ins, reason="G pads zeroed before gathers")

            # ================ LEVEL 3 (replicated) ================
            # pools for tail-weight prefetch, each spanning two levels
            st4 = contextlib.ExitStack()
            l4wp = st4.enter_context(tc.tile_pool(name="l4wp", bufs=1))
            l4st = st4.enter_context(tc.tile_pool(name="l4st", bufs=2))
            with tc.tile_pool(name="l3w", bufs=1) as wp, \
                 tc.tile_pool(name="l3wst", bufs=2) as wst, \
                 tc.tile_pool(name="l3p", bufs=1) as pp, \
                 tc.tile_pool(name="l3s", bufs=2) as ss, \
                 tc.tile_pool(name="l3m", bufs=4) as sm:
                w3c1_t = wload_blob(wp, wst, "w3c1_0")
                w3c2_t = [wload_blob(wp, wst, "w3c2_0"),
                          wload_blob(wp, wst, "w3c2_1")]
                # prefetch L4 weights (used next level)
                w4c1_t = [wload_blob(l4wp, l4st, "w4c1_0", dt=F32),
                          wload_blob(l4wp, l4st, "w4c1_1", dt=F32)]
                w4c2_t = [wload_blob(l4wp, l4st, "w4c2_0", dt=F32),
                          wload_blob(l4wp, l4st, "w4c2_1", dt=F32)]
                A3 = pp.tile([128, 12, 100], F32R)
                B3a = pp.tile([128, 10, 100], F32R)
                B3b = pp.tile([32, 10, 100], F32R)
                C3a = pp.tile([128, 512], F32R)
                C3b = pp.tile([32, 512], F32R)
                A3st = pp.tile([128, 12, 100], BF16)
                nc.vector.memset(B3a[:].bitcast(F32), 0.0)
                nc.vector.memset(B3b[:].bitcast(F32), 0.0)
                _r4 = nc.sync.dma_start(A3st[:],
                                        G3[:].rearrange("z c v -> c z v"))
                add_dep_helper(_r4.ins, ag3.ins, reason="gather before read")
                nc.scalar.copy(A3[:], A3st[:])

                # conv1
                for (z0, nz) in ((0, 8), (2, 8)):
                    N = nz * 64
                    for (c0, co_n) in ((0, 128), (128, 32)):
                        ps = pst.tile([co_n, 512], F32, tag="ps")
                        for o, (dz, dy, dx) in enumerate(OFFSETS):
                            w = A3[:].rearrange("p z (a b) -> p z a b", b=10)
                            nc.tensor.matmul(
                                ps[:, 0:N].rearrange(
                                    "p (z a b) -> p z a b", z=nz, a=8),
                                w3c1_t[:, o, c0:c0 + co_n],
                                w[:, z0 + dz + 1:z0 + dz + 1 + nz,
                                  1 + dy:9 + dy, 1 + dx:9 + dx],
                                start=(o == 0), stop=(o == 26))
                        mt = sm.tile([co_n, 512], F32, tag="m3mul")
                        nc.sync.dma_start(
                            mt[:, 0:N],
                            m3mul_d[z0:z0 + nz].flatten().unsqueeze(0)
                            .to_broadcast((co_n, N)))
                        B3 = B3a if c0 == 0 else B3b
                        dst = B3[:].rearrange("p z (a b) -> p z a b", b=10)
                        nc.vector.tensor_mul(
                            dst[:, z0:z0 + nz, 1:9, 1:9],
                            ps[:, 0:N].rearrange(
                                "p (z a b) -> p z a b", z=nz, a=8),
                            mt[:, 0:N].rearrange(
                                "p (z a b) -> p z a b", z=nz, a=8))

                # conv2
                for (c0, co_n) in ((0, 128), (128, 32)):
                    ps = pst.tile([co_n, 512], F32, tag="ps")
                    for o, (dz, dy, dx) in enumerate(OFFSETS):
                        for ki, B3 in enumerate((B3a, B3b)):
                            w = B3[:].rearrange("p z (a b) -> p z a b", b=10)
                            nc.tensor.matmul(
                                ps[:].rearrange("p (z a b) -> p z a b",
                                                z=8, a=8),
                                w3c2_t[ki][:, o, c0:c0 + co_n],
                                w[:, dz + 1:dz + 9, 1 + dy:9 + dy,
                                  1 + dx:9 + dx],
                                start=(o == 0 and ki == 0),
                                stop=(o == 26 and ki == 1))
                    C3 = C3a if c0 == 0 else C3b
                    if mf[3]:
                        mt = sm.tile([co_n, 512], F32, tag="mn3")
                        nc.sync.dma_start(mt[:], mn3_d[:].flatten().unsqueeze(0)
                                          .to_broadcast((co_n, 512)))
                        nc.vector.tensor_add(C3[:], ps[:], mt[:])
                    else:
                        nc.scalar.copy(C3[:], ps[:])

                # pool -> P4
                for C3, P4, cn in ((C3a, P4a, 128), (C3b, P4b, 32)):
                    v = C3[:].rearrange("p (z v) -> p z v", v=64)
                    t1 = ss.tile([cn, 4, 64], F32R, tag="pool3a")
                    nc.vector.tensor_max(t1[:], v[:, 0::2, :], v[:, 1::2, :])
                    u = t1[:].rearrange("p z (a b) -> p z a b", b=8)
                    t2 = ss.tile([cn, 4, 4, 8], F32R, tag="pool3b")
                    nc.vector.tensor_max(t2[:], u[:, :, 0::2, :],
                                         u[:, :, 1::2, :])
                    dst = P4[:].rearrange("p (z a b) -> p z a b", z=6, a=6)
                    if mf[4]:
                        t3 = ss.tile([cn, 4, 4, 4], F32R, tag="pool3c")
                        nc.vector.tensor_max(t3[:], t2[:, :, :, 0::2],
                                             t2[:, :, :, 1::2])
                        mt = sm.tile([cn, 64], F32, tag="m4p")
                        nc.sync.dma_start(mt[:], m4p_d[:].flatten().unsqueeze(0)
                                          .to_broadcast((cn, 64)))
                        nc.vector.tensor_mul(
                            dst[:, 1:5, 1:5, 1:5], t3[:],
                            mt[:].rearrange("p (z a b) -> p z a b", z=4, a=4))
                    else:
                        nc.vector.tensor_max(dst[:, 1:5, 1:5, 1:5],
                                             t2[:, :, :, 0::2],
                                             t2[:, :, :, 1::2])

                if DBG:
                    nc.sync.dma_start(dbg_d["dA3"][:], A3[:].bitcast(F32))
                    nc.sync.dma_start(dbg_d["dB3a"][:], B3a[:].bitcast(F32))
                    nc.sync.dma_start(dbg_d["dC3a"][:], C3a[:].bitcast(F32))

            # ================ TAIL (levels 4-6, replicated) ================
            def tail_conv(sm, wts, ins, outs, pg, og, mode, mdram, mname):
                N = og * og * og
                noff = wts[0].shape[1]
                offs = OFFSETS if noff == 27 else [(0, 0, 0)]
                for (ot, c0, co_n, padded) in outs:
                    ps = pst.tile([co_n, max(N, 8)], F32, tag="ps")
                    nmm = len(offs) * len(ins)
                    i = 0
                    for o, (dz, dy, dx) in enumerate(offs):
                        for ki, it in enumerate(ins):
                            w = it[:].rearrange("p (z a b) -> p z a b",
                                                z=pg, a=pg)
                            nc.tensor.matmul(
                                ps[:, 0:N].rearrange(
                                    "p (z a b) -> p z a b", z=og, a=og),
                                wts[ki][:, o, c0:c0 + co_n],
                                w[:, 1 + dz:1 + dz + og, 1 + dy:1 + dy + og,
                                  1 + dx:1 + dx + og],
                                start=(i == 0), stop=(i == nmm - 1))
                            i += 1
                    if padded:
                        opg = og + 2
                        dst = ot[:].rearrange("p (z a b) -> p z a b",
                                              z=opg, a=opg)[:, 1:1 + og,
                                                            1:1 + og, 1:1 + og]
                    else:
                        dst = ot[:, 0:N].rearrange("p (z a b) -> p z a b",
                                                   z=og, a=og)
                    src = ps[:, 0:N].rearrange("p (z a b) -> p z a b",
                                               z=og, a=og)
                    if mode == "copy":
                        nc.scalar.copy(dst, src)
                    else:
                        mt = sm.tile([co_n, N], F32, tag=mname)
                        nc.sync.dma_start(
                            mt[:], mdram[:].flatten().unsqueeze(0)
                            .to_broadcast((co_n, N)))
                        mm = mt[:].rearrange("p (z a b) -> p z a b", z=og, a=og)
                        if mode == "mul":
                            nc.vector.tensor_mul(dst, src, mm)
                        else:
                            nc.vector.tensor_add(dst, src, mm)

            def tail_pool(sm, ss, cs, ps_out, g, has_mask, mdram):
                go = g // 2
                for (ct, cn), (pt, _) in zip(cs, ps_out):
                    v = ct[:, 0:g * g * g].rearrange("p (z v) -> p z v",
                                                     v=g * g)
                    t1 = ss.tile([cn, go, g * g], F32, tag=f"tp{g}a")
                    nc.vector.tensor_max(t1[:], v[:, 0::2, :], v[:, 1::2, :])
                    u = t1[:].rearrange("p z (a b) -> p z a b", b=g)
                    t2 = ss.tile([cn, go, go, g], F32, tag=f"tp{g}b")
                    nc.vector.tensor_max(t2[:], u[:, :, 0::2, :],
                                         u[:, :, 1::2, :])
                    gp = go + 2
                    dst = pt[:].rearrange("p (z a b) -> p z a b", z=gp, a=gp)
                    if has_mask:
                        t3 = ss.tile([cn, go, go, go], F32, tag=f"tp{g}c")
                        nc.vector.tensor_max(t3[:], t2[:, :, :, 0::2],
                                             t2[:, :, :, 1::2])
                        mt = sm.tile([cn, go * go * go], F32, tag=f"tp{g}m")
                        nc.sync.dma_start(
                            mt[:], mdram[:].flatten().unsqueeze(0)
                            .to_broadcast((cn, go * go * go)))
                        nc.vector.tensor_mul(
                            dst[:, 1:1 + go, 1:1 + go, 1:1 + go], t3[:],
                            mt[:].rearrange("p (z a b) -> p z a b",
                                            z=go, a=go))
                    else:
                        nc.vector.tensor_max(
                            dst[:, 1:1 + go, 1:1 + go, 1:1 + go],
                            t2[:, :, :, 0::2], t2[:, :, :, 1::2])

            # ---- L4 ----
            with tc.tile_pool(name="l4p", bufs=1) as pp, \
                 tc.tile_pool(name="l4s", bufs=2) as ss, \
                 tc.tile_pool(name="l4m", bufs=2) as sm:
                B4a = pp.tile([128, 216], F32); B4b = pp.tile([64, 216], F32)
                C4a = pp.tile([128, 64], F32); C4b = pp.tile([64, 64], F32)
                nc.vector.memset(B4a[:].bitcast(F32), 0.0)
                nc.vector.memset(B4b[:].bitcast(F32), 0.0)
                tail_conv(sm, w4c1_t, [P4a, P4b],
                          [(B4a, 0, 128, True), (B4b, 128, 64, True)], 6, 4,
                          "mul" if mf[4] else "copy", m4mul_d, "m4mul")
                tail_conv(sm, w4c2_t, [B4a, B4b],
                          [(C4a, 0, 128, False), (C4b, 128, 64, False)], 6, 4,
                          "add" if mf[4] else "copy", mn4_d, "mn4")
                tail_pool(sm, ss, [(C4a, 128), (C4b, 64)],
                          [(P5a, 128), (P5b, 64)], 4, mf[5], m5p_d)

                if DBG:
                    nc.sync.dma_start(dbg_d["dP4a"][:], P4a[:])
                    nc.sync.dma_start(dbg_d["dB4a"][:], B4a[:])
                    nc.sync.dma_start(dbg_d["dC4a"][:], C4a[:])
            st4.close()

            # ---- L5 ----
            st5 = contextlib.ExitStack()
            wp = st5.enter_context(tc.tile_pool(name="l5w", bufs=1))
            wst = st5.enter_context(tc.tile_pool(name="l5wst", bufs=2))
            with tc.tile_pool(name="l5p", bufs=1) as pp, \
                 tc.tile_pool(name="l5s", bufs=2) as ss, \
                 tc.tile_pool(name="l5m", bufs=2) as sm:
                w5c1_t = [wload_blob(wp, wst, "w5c1_0", dt=F32),
                          wload_blob(wp, wst, "w5c1_1", dt=F32)]
                w5c2_t = [wload_blob(wp, wst, "w5c2_0", dt=F32),
                          wload_blob(wp, wst, "w5c2_1", dt=F32)]
                w6c1_t = [wload_blob(wp, wst, "w6c1_0", dt=F32),
                          wload_blob(wp, wst, "w6c1_1", dt=F32)]
                w6c2_t = [wload_blob(wp, wst, "w6c2_0", dt=F32),
                          wload_blob(wp, wst, "w6c2_1", dt=F32)]
                B5a = pp.tile([128, 64], F32); B5b = pp.tile([96, 64], F32)
                C5a = pp.tile([128, 8], F32); C5b = pp.tile([96, 8], F32)
                nc.vector.memset(B5a[:].bitcast(F32), 0.0)
                nc.vector.memset(B5b[:].bitcast(F32), 0.0)
                tail_conv(sm, w5c1_t, [P5a, P5b],
                          [(B5a, 0, 128, True), (B5b, 128, 96, True)], 4, 2,
                          "mul" if mf[5] else "copy", m5mul_d, "m5mul")
                tail_conv(sm, w5c2_t, [B5a, B5b],
                          [(C5a, 0, 128, False), (C5b, 128, 96, False)], 4, 2,
                          "add" if mf[5] else "copy", mn5_d, "mn5")
                tail_pool(sm, ss, [(C5a, 128), (C5b, 96)],
                          [(P6a, 128), (P6b, 96)], 2, mf[6], m6p_d)

                if DBG:
                    nc.sync.dma_start(dbg_d["dP5a"][:], P5a[:])
                    nc.sync.dma_start(dbg_d["dB5a"][:], B5a[:])
                    nc.sync.dma_start(dbg_d["dP6a"][:], P6a[:])

            # ---- L6 (1^3, center tap only) ----
            if True:
                for (ot, c0) in ((X6a, 0), (X6b, 128)):
                    ps = pst.tile([128, 8], F32, tag="ps")
                    nc.tensor.matmul(ps[:, 0:1], w6c1_t[0][:, 0, c0:c0 + 128],
                                     P6a[:, 13:14], start=True, stop=False)
                    nc.tensor.matmul(ps[:, 0:1], w6c1_t[1][:, 0, c0:c0 + 128],
                                     P6b[:, 13:14], start=False, stop=True)
                    nc.vector.tensor_copy(ot[:], ps[:, 0:1])
                for i, c0 in enumerate((0, 128)):
                    ps = pst.tile([128, 8], F32, tag="ps")
                    nc.tensor.matmul(ps[:, 0:1], w6c2_t[0][:, 0, c0:c0 + 128],
                                     X6a[:], start=True, stop=False)
                    nc.tensor.matmul(ps[:, 0:1], w6c2_t[1][:, 0, c0:c0 + 128],
                                     X6b[:], start=False, stop=True)
                    nc.scalar.copy(outt[:, i:i + 1], ps[:, 0:1])
            st5.close()
            if DBG:
                nc.sync.dma_start(dbg_d["dX6a"][:], X6a[:])
            nc.sync.dma_start(out_d[0, 0:128], outt[:, 0])
            nc.sync.dma_start(out_d[0, 128:256], outt[:, 1])

    nc.compile()
    return nc


_CACHE = {}


def kernel(features, coors, W0, W1, W2, W3, W4, W5, W6, W7, W8, W9, W10, W11,
           W12, W13):
    features = np.asarray(features, np.float32)
    coors = np.asarray(coors, np.int32)
    Ws = [np.asarray(w, np.float32) for w in
          (W0, W1, W2, W3, W4, W5, W6, W7, W8, W9, W10, W11, W12, W13)]
    in_maps, meta = build_host_inputs(features, coors, Ws)
    key = tuple(sorted(meta["mask_flags"].items()))
    if key not in _CACHE:
        _CACHE[key] = build_kernel(meta)
    nc = _CACHE[key]
    res = run_bass_kernel_spmd(nc, in_maps, core_ids=list(range(NC)))
    out = res.results[0]["out"].reshape(256)
    return out.reshape(1, 1, 1, 1, 256).astype(np.float32)


if __name__ == "__main__":
    pass
